# revision 14
# baseline (speedup 1.0000x reference)
"""Trainium2 Bass kernel for ConstructAdjMatrix (GNN message passing).

Math (reference):
    d_x = (rowsum(adj) + 1) ** -0.5          # [N_CELL]
    d_y = (colsum(adj) + 1) ** -0.5          # [N_DRUG]
    agg_cell_lp = d_x[:,None] * adj * d_y    # [N_CELL, N_DRUG]
    agg_drug_lp = agg_cell_lp.T              # [N_DRUG, N_CELL]
    self_cell_lp = diag(1/(rowsum+1) + 1)    # [N_CELL, N_CELL]
    self_drug_lp = diag(1/(colsum+1) + 1)    # [N_DRUG, N_DRUG]

Sharding: adj row-sharded across 8 cores (1024 rows each). Row degrees are
local; column degrees need an AllReduce across the 8 cores. Each core
writes its row block of agg_cell_lp and (via PE transpose) its column block
of agg_drug_lp. The diag outputs only need their diagonal vectors from the
device; the host assembles the (mostly zero) diag matrices.

To hide the AllReduce latency the columns are split into two halves, each
with its own pipelined AllReduce:
  - Left loads are prioritized (right loads dep on the matching left load),
    so the left column-sum AllReduce enters while right loads still stream.
  - All d_y-dependent work (out1 scale, transpose-stage scale) is split per
    half, so left-half writes start right after the left AllReduce, under
    the right AllReduce.
Per half: DVE accumulates acc += tile (accum_out = cumulative rowsums; the
per-tile rowsum falls out by difference), PE reduces acc over partitions
with a ones-vector matmul, and the d_y broadcast tile is built on-chip via
a K=2 ones matmul that also folds in the +1.
"""

import numpy as np

from concourse import bacc, bass, mybir, tile
from concourse.bass_utils import run_bass_kernel_spmd
from bass_rust import add_dep_helper

N_CELL, N_DRUG = 8192, 4096
NC = 8                 # cores
R = N_CELL // NC       # 1024 rows per core
P = 128                # partitions
NT = R // P            # 8 row tiles per core
HW = N_DRUG // 2       # 2048 column half-width
NBH = HW // 512        # 4 psum banks per half column-sum
NDH = HW // P          # 16 drug chunks of 128 per half
FP32 = mybir.dt.float32
ADD = mybir.AluOpType.add
MULT = mybir.AluOpType.mult
SUB = mybir.AluOpType.subtract


def _build_kernel():
    nc = bacc.Bacc(
        "TRN2", target_bir_lowering=False, debug=False, num_devices=NC
    )
    adj = nc.dram_tensor("adj_block", [R, N_DRUG], FP32, kind="ExternalInput").ap()
    out1 = nc.dram_tensor("out1", [R, N_DRUG], FP32, kind="ExternalOutput").ap()
    out2 = nc.dram_tensor("out2", [N_DRUG, R], FP32, kind="ExternalOutput").ap()
    dcell = nc.dram_tensor("dcell", [R], FP32, kind="ExternalOutput").ap()
    ddrug = nc.dram_tensor("ddrug", [N_DRUG], FP32, kind="ExternalOutput").ap()

    with tile.TileContext(nc) as tc:
        _body(tc, adj, out1, out2, dcell, ddrug)
    nc.compile()
    return nc


def _body(tc, adj, out1, out2, dcell, ddrug):
    nc = tc.nc
    from contextlib import ExitStack

    with ExitStack() as ctx:
        const = ctx.enter_context(tc.tile_pool(name="const", bufs=1))
        adj_pool = ctx.enter_context(tc.tile_pool(name="adjp", bufs=1))
        sb = ctx.enter_context(tc.tile_pool(name="sb", bufs=1))
        dram = ctx.enter_context(tc.tile_pool(name="dram", bufs=1, space="DRAM"))

        ident_dram = nc.inline_tensor(np.eye(P, dtype=np.float32), name="ident")
        identity = const.tile([P, P], FP32, name="identity")
        nc.sync.dma_start(out=identity[:], in_=ident_dram.ap())
        ones_col = const.tile([P, 1], FP32, name="ones_col")
        nc.vector.memset(ones_col[:], 1.0)
        ones2 = const.tile([2, P], FP32, name="ones2")
        nc.vector.memset(ones2[:], 1.0)

        cs_in = [dram.tile([HW], FP32, name=f"cs_in{h}") for h in range(2)]
        cs_out = [
            dram.tile([HW], FP32, name=f"cs_out{h}", addr_space="Shared")
            for h in range(2)
        ]

        cs_ctx = ExitStack()
        acc_pool = cs_ctx.enter_context(tc.tile_pool(name="accp", bufs=1))
        accs = [
            acc_pool.tile([P, HW], FP32, name=f"acc{h}", tag=f"acc{h}")
            for h in range(2)
        ]
        # cumulative rowsums per half, [128, NT]
        cums = [sb.tile([P, NT], FP32, name=f"cum{h}", tag=f"cum{h}") for h in range(2)]
        rrec = sb.tile([P, NT], FP32, name="rrec", tag="rrec")  # 1/(rowsum+1)
        dx = sb.tile([P, NT], FP32, name="dx", tag="dx")
        dc1 = sb.tile([P, NT], FP32, name="dc1", tag="dc1")

        # ---- Phase A: loads (left prioritized), acc, rowsums, dx prescale
        tiles = [[], []]  # [half][t]
        for t in range(NT):
            lt = adj_pool.tile([P, HW], FP32, name=f"aL{t}", tag=f"aL{t}")
            ld_l = nc.sync.dma_start(out=lt[:], in_=adj[t * P : (t + 1) * P, :HW])
            rt = adj_pool.tile([P, HW], FP32, name=f"aR{t}", tag=f"aR{t}")
            ld_r = nc.scalar.dma_start(out=rt[:], in_=adj[t * P : (t + 1) * P, HW:])
            # prioritize left loads: right load waits for the matching left
            add_dep_helper(ld_r.ins, ld_l.ins, sync=True, reason="stagger R after L")
            tiles[0].append(lt)
            tiles[1].append(rt)

        for h in range(2):
            for t in range(NT):
                at = tiles[h][t]
                if t == 0:
                    nc.vector.scalar_tensor_tensor(
                        out=accs[h][:], in0=at[:], scalar=0.0, in1=at[:],
                        op0=MULT, op1=ADD, accum_out=cums[h][:, 0:1],
                    )
                else:
                    nc.vector.scalar_tensor_tensor(
                        out=accs[h][:], in0=at[:], scalar=1.0, in1=accs[h][:],
                        op0=MULT, op1=ADD, accum_out=cums[h][:, t : t + 1],
                    )

        # rowsum_t = sum_h (cum_h[t] - cum_h[t-1]); then 1/(r+1), sqrt, +1
        nc.vector.tensor_tensor(out=rrec[:], in0=cums[0][:], in1=cums[1][:], op=ADD)
        nc.vector.tensor_tensor(
            out=rrec[:, 1:NT], in0=rrec[:, 1:NT], in1=rrec[:, 0 : NT - 1], op=SUB
        )
        nc.vector.tensor_scalar_add(rrec[:], rrec[:], 1.0)
        nc.vector.reciprocal(rrec[:], rrec[:])
        nc.scalar.sqrt(dx[:], rrec[:])
        nc.vector.tensor_scalar_add(dc1[:], rrec[:], 1.0)
        nc.sync.dma_start(out=dcell.rearrange("(t p) -> p t", p=P), in_=dc1[:])

        # dx prescale in place (ACT, per-partition scale)
        for t in range(NT):
            for h in range(2):
                nc.scalar.mul(tiles[h][t][:], tiles[h][t][:], dx[:, t : t + 1])

        # ---- Phase B: column sums -> two pipelined AllReduces ------------
        psum_cs = cs_ctx.enter_context(
            tc.tile_pool(name="psum_cs", bufs=1, space="PSUM")
        )
        csr_pool = cs_ctx.enter_context(tc.tile_pool(name="csrp", bufs=2))
        for h in range(2):
            cs_row = csr_pool.tile([1, HW], FP32, name=f"cs_row{h}", tag="cs_row")
            for b in range(NBH):
                csb = psum_cs.tile([1, 512], FP32, name=f"csb{h}_{b}", tag=f"csb{b}")
                nc.tensor.matmul(
                    csb[:1, :], ones_col[:], accs[h][:, b * 512 : (b + 1) * 512]
                )
                nc.any.tensor_copy(cs_row[:1, b * 512 : (b + 1) * 512], csb[:1, :])
            nc.sync.dma_start(out=cs_in[h][:], in_=cs_row[:1, :])
            nc.gpsimd.collective_compute(
                "AllReduce",
                mybir.AluOpType.add,
                replica_groups=[list(range(NC))],
                ins=[cs_in[h].opt()],
                outs=[cs_out[h].opt()],
            )
        cs_ctx.close()

        # ---- Phase C/D pools ---------------------------------------------
        out_ctx = ExitStack()
        late = out_ctx.enter_context(tc.tile_pool(name="late", bufs=1))
        psum_s = out_ctx.enter_context(tc.tile_pool(name="psum_s", bufs=1, space="PSUM"))
        psum_tp = out_ctx.enter_context(
            tc.tile_pool(name="psum_tp", bufs=4, space="PSUM")
        )
        o1_pool = out_ctx.enter_context(tc.tile_pool(name="o1p", bufs=2))
        stage_pool = out_ctx.enter_context(tc.tile_pool(name="stage", bufs=3))
        s2_pool = out_ctx.enter_context(tc.tile_pool(name="s2p", bufs=1))

        dy_ft = []   # f-major [128, 16] per half (per-partition scalars)
        dybs = []    # broadcast [128, HW] per half
        NFH = HW // P  # 16

        def dy_pipeline(h):
            # f-major: s_ft[p, f] = S[f*128 + p]
            s_ft = sb.tile([P, NFH], FP32, name=f"s_ft{h}", tag=f"s_ft{h}")
            nc.sync.dma_start(
                out=s_ft[:], in_=cs_out[h].rearrange("(f p) -> p f", p=P)
            )
            srec = sb.tile([P, NFH], FP32, name=f"srec{h}", tag=f"srec{h}")
            nc.vector.tensor_scalar_add(srec[:], s_ft[:], 1.0)
            nc.vector.reciprocal(srec[:], srec[:])
            dyf = sb.tile([P, NFH], FP32, name=f"dy_ft{h}", tag=f"dy_ft{h}")
            nc.scalar.sqrt(dyf[:], srec[:])
            dy_ft.append(dyf)
            dd1 = sb.tile([P, NFH], FP32, name=f"dd1{h}", tag=f"dd1{h}")
            nc.vector.tensor_scalar_add(dd1[:], srec[:], 1.0)
            nc.sync.dma_start(
                out=ddrug[h * HW : (h + 1) * HW].rearrange("(f p) -> p f", p=P),
                in_=dd1[:],
            )
            # broadcast (S+1) across partitions with a K=2 ones matmul,
            # then dyb = sqrt(1/(S+1)) in place
            s2 = s2_pool.tile([2, HW], FP32, name=f"s2_{h}", tag="s2")
            nc.vector.memset(s2[:2, :], 1.0)
            nc.sync.dma_start(out=s2[0:1, :], in_=cs_out[h][:])
            ps = psum_s.tile([P, HW], FP32, name=f"ps{h}", tag="ps")
            for j in range(NBH):
                nc.tensor.matmul(
                    ps[:, j * 512 : (j + 1) * 512],
                    ones2[:2, :],
                    s2[:2, j * 512 : (j + 1) * 512],
                )
            dyb = late.tile([P, HW], FP32, name=f"dyb{h}", tag=f"dyb{h}")
            nc.vector.reciprocal(dyb[:], ps[:])
            nc.scalar.sqrt(dyb[:], dyb[:])
            dybs.append(dyb)

        def out1_half(h):
            for t in range(NT):
                o1 = o1_pool.tile([P, HW], FP32, name=f"o1_{h}_{t}", tag="o1")
                nc.vector.tensor_tensor(
                    out=o1[:], in0=tiles[h][t][:], in1=dybs[h][:], op=MULT
                )
                nc.gpsimd.dma_start(
                    out=out1[t * P : (t + 1) * P, h * HW : (h + 1) * HW], in_=o1[:]
                )

        def out2_chunks(h, d_lo, d_hi):
            # drug chunks [d_lo, d_hi) within half h; global chunk = h*16 + d
            for d in range(d_lo, d_hi):
                gd = h * NDH + d
                stg = stage_pool.tile([P, R], FP32, name=f"stg{gd}", tag="stg")
                for g in range(2):
                    pt = psum_tp.tile([P, 512], FP32, name=f"pt{gd}_{g}", tag="ptp")
                    for t4 in range(4):
                        t = g * 4 + t4
                        nc.tensor.matmul(
                            pt[:, t4 * P : (t4 + 1) * P],
                            tiles[h][t][:, d * P : (d + 1) * P],
                            identity[:],
                            is_transpose=True,
                        )
                    nc.scalar.mul(
                        stg[:, g * 512 : (g + 1) * 512], pt[:], dy_ft[h][:, d : d + 1]
                    )
                nc.sync.dma_start(out=out2[gd * P : (gd + 1) * P, :], in_=stg[:])

        # Emission order tuned so each engine's in-order stream matches the
        # expected readiness times (left half first, right AR results land
        # while left writes stream).
        dy_pipeline(0)
        out1_half(0)
        out2_chunks(0, 0, 8)
        dy_pipeline(1)
        out2_chunks(0, 8, NDH)
        out1_half(1)
        out2_chunks(1, 0, NDH)
        out_ctx.close()


_CACHE = {}


def _get_kernel():
    if "nc" not in _CACHE:
        _CACHE["nc"] = _build_kernel()
    return _CACHE["nc"]


def kernel(adj):
    adj = np.ascontiguousarray(np.asarray(adj, dtype=np.float32))
    assert adj.shape == (N_CELL, N_DRUG)
    nc = _get_kernel()
    in_maps = [{"adj_block": adj[c * R : (c + 1) * R]} for c in range(NC)]
    res = run_bass_kernel_spmd(nc, in_maps, list(range(NC))).results

    agg_cell = np.concatenate([res[c]["out1"] for c in range(NC)], axis=0)
    agg_drug = np.concatenate([res[c]["out2"] for c in range(NC)], axis=1)
    self_cell = np.zeros((N_CELL, N_CELL), np.float32)
    np.fill_diagonal(self_cell, np.concatenate([res[c]["dcell"] for c in range(NC)]))
    self_drug = np.zeros((N_DRUG, N_DRUG), np.float32)
    np.fill_diagonal(self_drug, res[0]["ddrug"])
    return (agg_cell, agg_drug, self_cell, self_drug)


# revision 27
# speedup vs baseline: 1.0493x; 1.0493x over previous
"""Trainium2 Bass kernel for ConstructAdjMatrix (GNN message passing).

Math (reference):
    d_x = (rowsum(adj) + 1) ** -0.5          # [N_CELL]
    d_y = (colsum(adj) + 1) ** -0.5          # [N_DRUG]
    agg_cell_lp = d_x[:,None] * adj * d_y    # [N_CELL, N_DRUG]
    agg_drug_lp = agg_cell_lp.T              # [N_DRUG, N_CELL]
    self_cell_lp = diag(1/(rowsum+1) + 1)    # [N_CELL, N_CELL]
    self_drug_lp = diag(1/(colsum+1) + 1)    # [N_DRUG, N_DRUG]

Sharding: adj row-sharded across 8 cores (1024 rows each). Row degrees are
local; column degrees need an AllReduce across the 8 cores. Each core
writes its row block of agg_cell_lp and (via PE transpose) its column block
of agg_drug_lp. The diag outputs only need their diagonal vectors from the
device; the host assembles the (mostly zero) diag matrices.

To hide the AllReduce latency the columns are split into two halves, each
with its own pipelined AllReduce:
  - Left loads are prioritized (right loads dep on the matching left load),
    so the left column-sum AllReduce enters while right loads still stream.
  - All d_y-dependent work (out1 scale, transpose-stage scale) is split per
    half, so left-half writes start right after the left AllReduce, under
    the right AllReduce.
Per half: DVE accumulates acc += tile (accum_out = cumulative rowsums; the
per-tile rowsum falls out by difference), PE reduces acc over partitions
with a ones-vector matmul, and the d_y broadcast tile is built on-chip via
a K=2 ones matmul that also folds in the +1.
"""

import numpy as np

from concourse import bacc, bass, mybir, tile
from concourse.bass_utils import run_bass_kernel_spmd
from bass_rust import add_dep_helper

N_CELL, N_DRUG = 8192, 4096
NC = 8                 # cores
R = N_CELL // NC       # 1024 rows per core
P = 128                # partitions
NT = R // P            # 8 row tiles per core
HW = N_DRUG // 2       # 2048 column half-width
NBH = HW // 512        # 4 psum banks per half column-sum
NDH = HW // P          # 16 drug chunks of 128 per half
FP32 = mybir.dt.float32
ADD = mybir.AluOpType.add
MULT = mybir.AluOpType.mult
SUB = mybir.AluOpType.subtract


def _build_kernel():
    nc = bacc.Bacc(
        "TRN2", target_bir_lowering=False, debug=False, num_devices=NC
    )
    adj = nc.dram_tensor("adj_block", [R, N_DRUG], FP32, kind="ExternalInput").ap()
    out1 = nc.dram_tensor("out1", [R, N_DRUG], FP32, kind="ExternalOutput").ap()
    out2 = nc.dram_tensor("out2", [N_DRUG, R], FP32, kind="ExternalOutput").ap()
    dcell = nc.dram_tensor("dcell", [R], FP32, kind="ExternalOutput").ap()
    ddrug = nc.dram_tensor("ddrug", [N_DRUG], FP32, kind="ExternalOutput").ap()

    with tile.TileContext(nc) as tc:
        _body(tc, adj, out1, out2, dcell, ddrug)
    nc.compile()
    return nc


def _body(tc, adj, out1, out2, dcell, ddrug):
    nc = tc.nc
    from contextlib import ExitStack

    with ExitStack() as ctx:
        const = ctx.enter_context(tc.tile_pool(name="const", bufs=1))
        adj_pool = ctx.enter_context(tc.tile_pool(name="adjp", bufs=1))
        sb = ctx.enter_context(tc.tile_pool(name="sb", bufs=1))
        dram = ctx.enter_context(tc.tile_pool(name="dram", bufs=1, space="DRAM"))

        ident_dram = nc.inline_tensor(np.eye(P, dtype=np.float32), name="ident")
        identity = const.tile([P, P], FP32, name="identity")
        nc.sync.dma_start(out=identity[:], in_=ident_dram.ap())
        ones_col = const.tile([P, 1], FP32, name="ones_col")
        nc.vector.memset(ones_col[:], 1.0)
        ones2 = const.tile([2, P], FP32, name="ones2")
        nc.vector.memset(ones2[:], 1.0)
        # shared K=2 rhs for the d_y broadcast: row 1 stays 1.0, row 0 gets
        # the AllReduce result per half (the +1 folds into the matmul)
        s2 = const.tile([2, HW], FP32, name="s2")
        nc.vector.memset(s2[:2, :], 1.0)

        cs_in = [dram.tile([HW], FP32, name=f"cs_in{h}") for h in range(2)]
        cs_out = [
            dram.tile([HW], FP32, name=f"cs_out{h}", addr_space="Shared")
            for h in range(2)
        ]

        acc_pool = ctx.enter_context(tc.tile_pool(name="accp", bufs=1))
        accs = [
            acc_pool.tile([P, HW], FP32, name=f"acc{h}", tag=f"acc{h}")
            for h in range(2)
        ]
        # cumulative rowsums per half, [128, NT]
        cums = [sb.tile([P, NT], FP32, name=f"cum{h}", tag=f"cum{h}") for h in range(2)]
        rrec = sb.tile([P, NT], FP32, name="rrec", tag="rrec")  # 1/(rowsum+1)
        dx = sb.tile([P, NT], FP32, name="dx", tag="dx")
        dc1 = sb.tile([P, NT], FP32, name="dc1", tag="dc1")

        # ---- Phase A: loads (left prioritized), acc, rowsums, dx prescale
        tiles = [[], []]  # [half][t]
        for t in range(NT):
            lt = adj_pool.tile([P, HW], FP32, name=f"aL{t}", tag=f"aL{t}")
            ld_l = nc.sync.dma_start(out=lt[:], in_=adj[t * P : (t + 1) * P, :HW])
            rt = adj_pool.tile([P, HW], FP32, name=f"aR{t}", tag=f"aR{t}")
            ld_r = nc.scalar.dma_start(out=rt[:], in_=adj[t * P : (t + 1) * P, HW:])
            # prioritize left loads: right load waits for the matching left
            add_dep_helper(ld_r.ins, ld_l.ins, sync=True, reason="stagger R after L")
            tiles[0].append(lt)
            tiles[1].append(rt)

        # Shared PSUM pool: "ptp" tag (4 banks) serves the column-sum matmuls
        # first, then the transposes; "ps" tag (4 banks) serves the two d_y
        # broadcast matmul groups.
        psum = ctx.enter_context(tc.tile_pool(name="psum", bufs=1, space="PSUM"))
        csr_pool = ctx.enter_context(tc.tile_pool(name="csrp", bufs=2))

        def acc_half(h):
            for t in range(NT):
                at = tiles[h][t]
                if t == 0:
                    nc.vector.scalar_tensor_tensor(
                        out=accs[h][:], in0=at[:], scalar=0.0, in1=at[:],
                        op0=MULT, op1=ADD, accum_out=cums[h][:, 0:1],
                    )
                else:
                    nc.vector.scalar_tensor_tensor(
                        out=accs[h][:], in0=at[:], scalar=1.0, in1=accs[h][:],
                        op0=MULT, op1=ADD, accum_out=cums[h][:, t : t + 1],
                    )

        def colsum_allreduce(h, copy_fn):
            cs_row = csr_pool.tile([1, HW], FP32, name=f"cs_row{h}", tag="cs_row")
            for b in range(NBH):
                csb = psum.tile([P, 512], FP32, name=f"csb{h}_{b}", tag="ptp", bufs=4)
                nc.tensor.matmul(
                    csb[:1, :], ones_col[:], accs[h][:, b * 512 : (b + 1) * 512]
                )
                copy_fn(cs_row[:1, b * 512 : (b + 1) * 512], csb[:1, :])
            nc.sync.dma_start(out=cs_in[h][:], in_=cs_row[:1, :])
            nc.gpsimd.collective_compute(
                "AllReduce",
                mybir.AluOpType.add,
                replica_groups=[list(range(NC))],
                ins=[cs_in[h].opt()],
                outs=[cs_out[h].opt()],
            )

        # left half: acc chain then its AllReduce immediately
        acc_half(0)
        colsum_allreduce(0, nc.scalar.copy)
        acc_half(1)

        # rowsum_t = sum_h (cum_h[t] - cum_h[t-1]); then 1/(r+1), sqrt, +1
        nc.vector.tensor_tensor(out=rrec[:], in0=cums[0][:], in1=cums[1][:], op=ADD)
        nc.vector.tensor_tensor(
            out=rrec[:, 1:NT], in0=rrec[:, 1:NT], in1=rrec[:, 0 : NT - 1], op=SUB
        )
        nc.vector.tensor_scalar_add(rrec[:], rrec[:], 1.0)
        nc.vector.reciprocal(rrec[:], rrec[:])
        nc.scalar.sqrt(dx[:], rrec[:])
        nc.vector.tensor_scalar_add(dc1[:], rrec[:], 1.0)

        # ---- Phase C/D pools ---------------------------------------------
        late = ctx.enter_context(tc.tile_pool(name="late", bufs=1))
        stage_pool = ctx.enter_context(tc.tile_pool(name="stage", bufs=3))

        dybs = []    # broadcast [128, HW] per half
        NFH = HW // P  # 16

        def dy_pipeline(h, dma):
            # broadcast (S+1) across partitions with a K=2 ones matmul,
            # then dyb = sqrt(1/(S+1)) in place
            dma.dma_start(out=s2[0:1, :], in_=cs_out[h][:])
            ps = psum.tile([P, HW], FP32, name=f"ps{h}", tag="ps")
            for j in range(NBH):
                nc.tensor.matmul(
                    ps[:, j * 512 : (j + 1) * 512],
                    ones2[:2, :],
                    s2[:2, j * 512 : (j + 1) * 512],
                )
            dyb = late.tile([P, HW], FP32, name=f"dyb{h}", tag=f"dyb{h}")
            nc.vector.reciprocal(dyb[:], ps[:])
            nc.scalar.sqrt(dyb[:], dyb[:])
            dybs.append(dyb)

        def scale_store_out1(h, t, dma):
            # tile becomes the final out1 block: tile = (tile * d_x) * d_y
            at = tiles[h][t]
            nc.vector.scalar_tensor_tensor(
                out=at[:], in0=at[:], scalar=dx[:, t : t + 1], in1=dybs[h][:],
                op0=MULT, op1=MULT,
            )
            dma.dma_start(
                out=out1[t * P : (t + 1) * P, h * HW : (h + 1) * HW], in_=at[:]
            )

        def ddrug_half(h, dma):
            # self_drug diag values 1/(S+1)+1, in f-major layout
            s_ft = sb.tile([P, NFH], FP32, name=f"s_ft{h}", tag=f"s_ft{h}")
            dma.dma_start(out=s_ft[:], in_=cs_out[h].rearrange("(f p) -> p f", p=P))
            nc.vector.tensor_scalar_add(s_ft[:], s_ft[:], 1.0)
            nc.vector.reciprocal(s_ft[:], s_ft[:])
            nc.vector.tensor_scalar_add(s_ft[:], s_ft[:], 1.0)
            dma.dma_start(
                out=ddrug[h * HW : (h + 1) * HW].rearrange("(f p) -> p f", p=P),
                in_=s_ft[:],
            )

        def out2_chunks(h, d_lo, d_hi):
            # drug chunks [d_lo, d_hi) within half h; global chunk = h*16 + d.
            # tiles are fully scaled already, so the PSUM->SBUF copy is plain.
            for d in range(d_lo, d_hi):
                gd = h * NDH + d
                stg = stage_pool.tile([P, R], FP32, name=f"stg{gd}", tag="stg")
                for g in range(2):
                    pt = psum.tile([P, 512], FP32, name=f"pt{gd}_{g}", tag="ptp", bufs=4)
                    for t4 in range(4):
                        t = g * 4 + t4
                        nc.tensor.matmul(
                            pt[:, t4 * P : (t4 + 1) * P],
                            tiles[h][t][:, d * P : (d + 1) * P],
                            identity[:],
                            is_transpose=True,
                        )
                    nc.scalar.copy(stg[:, g * 512 : (g + 1) * 512], pt[:])
                nc.scalar.dma_start(out=out2[gd * P : (gd + 1) * P, :], in_=stg[:])

        # Emission order tuned so each engine's in-order stream matches the
        # expected readiness times: left dyb right after AR-L, then the right
        # AllReduce, left out1 scales/stores while AR-R is in flight, right
        # half following, transposes/stage stores last (they pace the tail).
        dy_pipeline(0, nc.sync)
        colsum_allreduce(1, nc.scalar.copy)
        nc.sync.dma_start(out=dcell.rearrange("(t p) -> p t", p=P), in_=dc1[:])
        for t in range(NT):
            scale_store_out1(0, t, nc.sync)
        dy_pipeline(1, nc.gpsimd)
        for t in range(NT):
            scale_store_out1(1, t, nc.gpsimd)
        ddrug_half(0, nc.sync)
        ddrug_half(1, nc.gpsimd)
        out2_chunks(0, 0, NDH)
        out2_chunks(1, 0, NDH)


_CACHE = {}


def _get_kernel():
    if "nc" not in _CACHE:
        _CACHE["nc"] = _build_kernel()
    return _CACHE["nc"]


def kernel(adj):
    adj = np.ascontiguousarray(np.asarray(adj, dtype=np.float32))
    assert adj.shape == (N_CELL, N_DRUG)
    nc = _get_kernel()
    in_maps = [{"adj_block": adj[c * R : (c + 1) * R]} for c in range(NC)]
    res = run_bass_kernel_spmd(nc, in_maps, list(range(NC))).results

    agg_cell = np.concatenate([res[c]["out1"] for c in range(NC)], axis=0)
    agg_drug = np.concatenate([res[c]["out2"] for c in range(NC)], axis=1)
    self_cell = np.zeros((N_CELL, N_CELL), np.float32)
    np.fill_diagonal(self_cell, np.concatenate([res[c]["dcell"] for c in range(NC)]))
    self_drug = np.zeros((N_DRUG, N_DRUG), np.float32)
    np.fill_diagonal(self_drug, res[0]["ddrug"])
    return (agg_cell, agg_drug, self_cell, self_drug)
